# revision 7
# baseline (speedup 1.0000x reference)
"""Trainium2 Bass kernel for nn_Model_17291538333963 (gnn_message_passing).

Structure of the model (B=16, T=96, N=H=512, MT=192, C4=768):
  - The GRU runs on x*0 (zeros!), so the whole GRU -> attention -> Laplacian ->
    Chebyshev chain depends ONLY on weights and is identical across batch.
    It is computed once on host (pure weight preprocessing).
  - The length-4 FFT (with cheb[0] == 0) and irfft collapse to small linear
    combinations which are folded into the surrounding weight matrices.
  - The remaining work is a dense, batch-parallel pipeline:
       P = combos(L) @ Xp        (4 streams a,c,e,d)
       GLU-A (288/96 -> 768), GLU-B1 (768 -> 768), GLU-B2 (768 -> 576/192)
       head: igft (fold irfft+weight_param), fl+sigmoid, frl, fc1+leaky, fc2
    which is sharded data-parallel over batch: 2 batch items per core x 8.
  - All activations flow through the device in TRANSPOSED layout
    (features on partitions, nodes on the free dim), so every matmul is
    out[p,f] = sum_c lhsT[c,p] rhs[c,f] with host-pretransposed weights.
"""

import os
import numpy as np
import ml_dtypes

B, T, N = 16, 96, 512
MT = 192
C4 = 768
NCORES = 8
BPC = B // NCORES  # 2 batch items per core

BF16 = ml_dtypes.bfloat16

# cached compiled program + dram tensor handles
_CACHE = {}
LAST_RESULTS = None  # BassKernelResults of the most recent run (for profiling)


# ----------------------------------------------------------------------------
# host-side precompute (weight-only math + layout packing)
# ----------------------------------------------------------------------------

def _sigmoid(x):
    out = np.empty_like(x)
    np.negative(x, out=out)
    np.exp(out, out=out)
    out += 1.0
    np.reciprocal(out, out=out)
    return out


def _host_precompute(inp):
    """All math that depends only on weights; returns packed device tensors
    (shared across cores) plus the replicated attention matrix output."""
    H = N
    f8 = np.float64
    # --- GRU over 512 steps with zero input, single row (batch-identical) ---
    b_ih = inp["b_ih"].astype(f8)
    W_hh = inp["W_hh"].astype(f8)
    b_hh = inp["b_hh"].astype(f8)
    xr, xz, xn = b_ih[:H], b_ih[H:2 * H], b_ih[2 * H:]
    h = np.zeros(H, f8)
    outs = np.empty((N, H), f8)
    for t in range(N):
        gh = W_hh @ h + b_hh
        r = _sigmoid(xr + gh[:H])
        z = _sigmoid(xz + gh[H:2 * H])
        nn_ = np.tanh(xn + r * gh[2 * H:])
        h = (1.0 - z) * nn_ + z * h
        outs[t] = h
    # --- attention (identical for every batch element) ---
    key = outs.T @ inp["wk"].astype(f8)[:, 0]
    qry = outs.T @ inp["wq"].astype(f8)[:, 0]
    e = key[:, None] + qry[None, :]
    e = np.where(e > 0, e, 0.2 * e)
    e -= e.max(axis=1, keepdims=True)
    att = np.exp(e)
    att /= att.sum(axis=1, keepdims=True)
    # --- laplacian + chebyshev combos ---
    deg = att.sum(axis=1)
    att_sym = 0.5 * (att + att.T)
    d_inv = 1.0 / (np.sqrt(deg) + 1e-7)
    lap = d_inv[:, None] * (np.diag(deg) - att_sym) * d_inv[None, :]
    L2 = lap
    L3 = 2.0 * lap @ L2
    L4 = 2.0 * lap @ L3 - L2
    # gft streams: a = g1+g2+g3, c = -g2, e = -g1+g2-g3, d = g3-g1
    Lmats = np.stack([L2 + L3 + L4, -L3, -L2 + L3 - L4, L4 - L2]).astype(np.float32)

    dev = {}
    # lmT[p, j, c, n] = Lmats[j].T[c*128+p, n]
    lmT = np.ascontiguousarray(
        Lmats.transpose(0, 2, 1).reshape(4, 4, 128, N).transpose(2, 0, 1, 3))
    dev["lmT"] = lmT.astype(BF16)

    # --- GLU-A folded weights ---
    def fold_real(W):  # (C4, 4T) -> (C4, 3T): [W0, W1+W3, W2]
        W0, W1, W2, W3 = W[:, :T], W[:, T:2 * T], W[:, 2 * T:3 * T], W[:, 3 * T:]
        return np.concatenate([W0, W1 + W3, W2], axis=1)

    def fold_imag(W):  # imag input = [0, d, 0, -d] -> W1 - W3
        return W[:, T:2 * T] - W[:, 3 * T:]

    # wa_re[p, proj, ci, o] = W_eff[o, ci*96+p]
    wa_re = np.stack([fold_real(inp["gluA_lW"][0]), fold_real(inp["gluA_rW"][0])])
    dev["wa_re"] = np.ascontiguousarray(
        wa_re.transpose(2, 0, 1).reshape(3, 96, 2, C4).transpose(1, 2, 0, 3)
    ).astype(BF16)
    wa_im = np.stack([fold_imag(inp["gluA_lW"][1]), fold_imag(inp["gluA_rW"][1])])
    dev["wa_im"] = np.ascontiguousarray(wa_im.transpose(2, 0, 1)).astype(BF16)

    # wb1[p, mi, ci, o] = Wmi[o, ci*128+p]
    wb1 = np.stack([inp["gluB_lW"][0], inp["gluB_rW"][0],
                    inp["gluB_lW"][1], inp["gluB_rW"][1]])  # (4, 768, 768)
    dev["wb1"] = np.ascontiguousarray(
        wb1.transpose(2, 0, 1).reshape(6, 128, 4, C4).transpose(1, 2, 0, 3)
    ).astype(BF16)

    wb2r = np.stack([inp["gluB_lW"][2][:576], inp["gluB_rW"][2][:576]])  # (2,576,768)
    dev["wb2r"] = np.ascontiguousarray(
        wb2r.transpose(2, 0, 1).reshape(6, 128, 2, 576).transpose(1, 2, 0, 3)
    ).astype(BF16)
    wb2i = np.stack([inp["gluB_lW"][3][192:384], inp["gluB_rW"][3][192:384]])
    dev["wb2i"] = np.ascontiguousarray(
        wb2i.transpose(2, 0, 1).reshape(6, 128, 2, MT).transpose(1, 2, 0, 3)
    ).astype(BF16)

    # --- head: fold irfft + weight_param ---
    Wp = inp["weight_param"].astype(f8)
    Ms = np.stack([(Wp[0] + Wp[1] + Wp[2] + Wp[3]) / 4.0,
                   (Wp[0] - Wp[2]) / 2.0,
                   (Wp[0] - Wp[1] + Wp[2] - Wp[3]) / 4.0,
                   (Wp[3] - Wp[1]) / 2.0]).astype(np.float32)  # (4, 192, 192)
    # wms[c, j, tb, ub, u] = Ms[j][tb*96+c, ub*96+u]
    dev["wms"] = np.ascontiguousarray(
        Ms.reshape(4, 2, 96, 2, 96).transpose(2, 0, 1, 3, 4))
    # wfl[c, cb, ob, u] = fl_W[ob*96+u, cb*96+c]
    flW = inp["fl_W"].astype(np.float32)  # (192, 192)
    dev["wfl"] = np.ascontiguousarray(
        flW.reshape(2, 96, 2, 96).transpose(3, 2, 0, 1))
    # wfrl[c, cb, u] = frl_W[u, cb*96+c]
    dev["wfrl"] = np.ascontiguousarray(
        inp["frl_W"].astype(np.float32).reshape(T, 2, 96).transpose(2, 1, 0))
    dev["wfc1"] = np.ascontiguousarray(inp["fc1_W"].astype(np.float32).T)  # (96, 96)
    dev["wfc2"] = np.ascontiguousarray(inp["fc2_W"].astype(np.float32).T)  # (96, 1)

    # --- biases ---
    ba = np.stack([np.stack([inp["gluA_lb"][0], inp["gluA_rb"][0]]),
                   np.stack([inp["gluA_lb"][1], inp["gluA_rb"][1]])])  # (2,2,768)
    dev["ba"] = np.ascontiguousarray(
        ba.reshape(2, 2, 6, 128).transpose(3, 0, 1, 2)).astype(np.float32)
    bb1 = np.stack([inp["gluB_lb"][0], inp["gluB_rb"][0],
                    inp["gluB_lb"][1], inp["gluB_rb"][1]])  # (4, 768)
    dev["bb1"] = np.ascontiguousarray(
        bb1.reshape(4, 6, 128).transpose(2, 0, 1)).astype(np.float32)
    # bb2[p, proj, oc]: oc 0..5 real chunks of 96, oc 6..7 imag chunks
    bb2 = np.empty((96, 2, 8), np.float32)
    for pi, (br, bi) in enumerate([(inp["gluB_lb"][2], inp["gluB_lb"][3]),
                                   (inp["gluB_rb"][2], inp["gluB_rb"][3])]):
        bb2[:, pi, :6] = np.asarray(br[:576]).reshape(6, 96).T
        bb2[:, pi, 6:] = np.asarray(bi[192:384]).reshape(2, 96).T
    dev["bb2"] = bb2
    bhd = np.empty((96, 4), np.float32)
    bhd[:, 0:2] = np.asarray(inp["fl_b"]).reshape(2, 96).T
    bhd[:, 2] = np.asarray(inp["frl_b"])
    bhd[:, 3] = np.asarray(inp["fc1_b"])
    dev["bhd"] = bhd
    dev["bfc2"] = np.asarray(inp["fc2_b"]).astype(np.float32).reshape(1, 1)

    return dev, att.astype(np.float32)


def _pack_x(x):
    """x (B, T, N) -> per-core shards (BPC, 128, 4, T) bf16 with
    [b, p, c, t] = x[b][t, c*128+p]."""
    xt = np.ascontiguousarray(
        x.transpose(0, 2, 1).reshape(B, 4, 128, T).transpose(0, 2, 1, 3))
    xt = xt.astype(BF16)
    return [xt[i * BPC:(i + 1) * BPC] for i in range(NCORES)]


# ----------------------------------------------------------------------------
# device program
# ----------------------------------------------------------------------------

def _build_program(dev_shapes):
    import concourse.bass as bass  # noqa: F401
    import concourse.mybir as mybir
    import concourse.tile as tile
    from concourse import bacc

    f32 = mybir.dt.float32
    f32r = mybir.dt.float32r
    bf16 = mybir.dt.bfloat16
    AF = mybir.ActivationFunctionType
    OP = mybir.AluOpType

    nc = bacc.Bacc("TRN2", target_bir_lowering=False)

    d = {}
    d["xt"] = nc.dram_tensor("xt", (BPC, 128, 4, T), bf16, kind="ExternalInput")
    for name, arr_shape, dt_ in dev_shapes:
        d[name] = nc.dram_tensor(name, arr_shape, dt_, kind="ExternalInput")
    d_out = nc.dram_tensor("out", (BPC, N), f32, kind="ExternalOutput")

    with tile.TileContext(nc) as tc:
        with (
            tc.tile_pool(name="wp", bufs=1) as wp,
            tc.tile_pool(name="xpool", bufs=2) as xpool,
            tc.tile_pool(name="ptp", bufs=2) as ptp,
            tc.tile_pool(name="apool", bufs=2) as apool,
            tc.tile_pool(name="bpool", bufs=2) as bpool,
            tc.tile_pool(name="rpool", bufs=1) as rpool,
            tc.tile_pool(name="hp", bufs=1) as hp,
            tc.tile_pool(name="sgp", bufs=4) as sgp,
            tc.tile_pool(name="pspt", bufs=3, space="PSUM") as pspt,
            tc.tile_pool(name="psglu", bufs=2, space="PSUM") as psglu,
        ):
            # ---- resident weights ----
            w = {}
            wt_specs = [
                ("lmT", [128, 4, 4, N], bf16),
                ("wa_re", [96, 2, 3, C4], bf16),
                ("wa_im", [96, 2, C4], bf16),
                ("wb1", [128, 4, 6, C4], bf16),
                ("wb2r", [128, 2, 6, 576], bf16),
                ("wb2i", [128, 2, 6, MT], bf16),
                ("wms", [96, 4, 2, 2, 96], f32r),
                ("wfl", [96, 2, 2, 96], f32r),
                ("wfrl", [96, 2, 96], f32r),
                ("wfc1", [96, 96], f32r),
                ("wfc2", [96, 1], f32r),
                ("ba", [128, 2, 2, 6], f32),
                ("bb1", [128, 4, 6], f32),
                ("bb2", [96, 2, 8], f32),
                ("bhd", [96, 4], f32),
                ("bfc2", [1, 1], f32),
            ]
            for name, shp, dt_ in wt_specs:
                w[name] = wp.tile(shp, dt_, tag=name, name=name)
                nc.sync.dma_start(out=w[name], in_=d[name][:])

            for b in range(BPC):
                # ---- chebyshev streams: P^T[j] (96 x 512), j in {a,c,e,d} ----
                xp = xpool.tile([128, 4, T], bf16, tag="xp")
                nc.sync.dma_start(out=xp, in_=d["xt"][b])
                pts = ptp.tile([96, 4, N], bf16, tag="pts")
                for j in range(4):
                    ps = pspt.tile([96, N], f32, tag="ps_pt")
                    for c in range(4):
                        nc.tensor.matmul(ps, lhsT=xp[:, c, :], rhs=w["lmT"][:, j, c, :],
                                         start=(c == 0), stop=(c == 3))
                    nc.vector.tensor_copy(pts[:, j, :], ps)

                # ---- GLU A ----
                realA = apool.tile([128, 6, N], bf16, tag="realA")
                imagA = apool.tile([128, 6, N], bf16, tag="imagA")
                for path in range(2):
                    ncis = 3 if path == 0 else 1
                    dst = realA if path == 0 else imagA
                    for oi in range(6):
                        psl = psglu.tile([128, N], f32, tag="psl")
                        psr = psglu.tile([128, N], f32, tag="psr")
                        for ci in range(ncis):
                            rhs = pts[:, ci, :] if path == 0 else pts[:, 3, :]
                            if path == 0:
                                ll = w["wa_re"][:, 0, ci, oi * 128:(oi + 1) * 128]
                                lr = w["wa_re"][:, 1, ci, oi * 128:(oi + 1) * 128]
                            else:
                                ll = w["wa_im"][:, 0, oi * 128:(oi + 1) * 128]
                                lr = w["wa_im"][:, 1, oi * 128:(oi + 1) * 128]
                            nc.tensor.matmul(psl, lhsT=ll, rhs=rhs,
                                             start=(ci == 0), stop=(ci == ncis - 1))
                            nc.tensor.matmul(psr, lhsT=lr, rhs=rhs,
                                             start=(ci == 0), stop=(ci == ncis - 1))
                        sig = sgp.tile([128, N], f32, tag="sig")
                        nc.scalar.activation(sig, psr, AF.Sigmoid,
                                             bias=w["ba"][:, path, 1, oi:oi + 1])
                        nc.vector.scalar_tensor_tensor(
                            out=dst[:, oi, :], in0=psl,
                            scalar=w["ba"][:, path, 0, oi:oi + 1], in1=sig,
                            op0=OP.add, op1=OP.mult)

                # ---- GLU B1 ----
                realB = bpool.tile([128, 6, N], bf16, tag="realB")
                imagB = bpool.tile([128, 6, N], bf16, tag="imagB")
                for path in range(2):
                    src = realA if path == 0 else imagA
                    dst = realB if path == 0 else imagB
                    ml, mr = (0, 1) if path == 0 else (2, 3)
                    for oi in range(6):
                        psl = psglu.tile([128, N], f32, tag="psl")
                        psr = psglu.tile([128, N], f32, tag="psr")
                        for ci in range(6):
                            rhs = src[:, ci, :]
                            nc.tensor.matmul(
                                psl, lhsT=w["wb1"][:, ml, ci, oi * 128:(oi + 1) * 128],
                                rhs=rhs, start=(ci == 0), stop=(ci == 5))
                            nc.tensor.matmul(
                                psr, lhsT=w["wb1"][:, mr, ci, oi * 128:(oi + 1) * 128],
                                rhs=rhs, start=(ci == 0), stop=(ci == 5))
                        sig = sgp.tile([128, N], f32, tag="sig")
                        nc.scalar.activation(sig, psr, AF.Sigmoid,
                                             bias=w["bb1"][:, mr, oi:oi + 1])
                        nc.vector.scalar_tensor_tensor(
                            out=dst[:, oi, :], in0=psl,
                            scalar=w["bb1"][:, ml, oi:oi + 1], in1=sig,
                            op0=OP.add, op1=OP.mult)

                # ---- GLU B2 (only the output chunks the head consumes) ----
                Rsb = rpool.tile([96, 6, N], f32r, tag="Rsb")
                Isb = rpool.tile([96, 2, N], f32r, tag="Isb")
                for path in range(2):
                    noc = 6 if path == 0 else 2
                    src = realB if path == 0 else imagB
                    wgt = w["wb2r"] if path == 0 else w["wb2i"]
                    dst = Rsb if path == 0 else Isb
                    for oc in range(noc):
                        psl = psglu.tile([96, N], f32, tag="psl")
                        psr = psglu.tile([96, N], f32, tag="psr")
                        for ci in range(6):
                            rhs = src[:, ci, :]
                            nc.tensor.matmul(
                                psl, lhsT=wgt[:, 0, ci, oc * 96:(oc + 1) * 96],
                                rhs=rhs, start=(ci == 0), stop=(ci == 5))
                            nc.tensor.matmul(
                                psr, lhsT=wgt[:, 1, ci, oc * 96:(oc + 1) * 96],
                                rhs=rhs, start=(ci == 0), stop=(ci == 5))
                        boff = oc if path == 0 else 6 + oc
                        sig = sgp.tile([96, N], f32, tag="sig")
                        nc.scalar.activation(sig, psr, AF.Sigmoid,
                                             bias=w["bb2"][:, 1, boff:boff + 1])
                        nc.vector.scalar_tensor_tensor(
                            out=dst[:, oc, :], in0=psl,
                            scalar=w["bb2"][:, 0, boff:boff + 1], in1=sig,
                            op0=OP.add, op1=OP.mult)

                # ---- head (float32r matmuls) ----
                def mmr(ps, lhsT, rhs, start, stop):
                    nc.tensor.matmul(ps, lhsT=lhsT, rhs=rhs, start=start, stop=stop)

                igft = hp.tile([96, 2, N], f32r, tag="igft")
                for ub in range(2):
                    ps = pspt.tile([96, N], f32, tag="ps_pt")
                    k = 0
                    for j in range(3):
                        for tb in range(2):
                            mmr(ps, w["wms"][:, j, tb, ub, :], Rsb[:, 2 * j + tb, :],
                                k == 0, k == 7)
                            k += 1
                    for tb in range(2):
                        mmr(ps, w["wms"][:, 3, tb, ub, :], Isb[:, tb, :], k == 0, k == 7)
                        k += 1
                    nc.scalar.copy(igft[:, ub, :], ps)

                src_sb = hp.tile([96, 2, N], f32r, tag="srcsb")
                for ob in range(2):
                    ps = pspt.tile([96, N], f32, tag="ps_pt")
                    for cb in range(2):
                        mmr(ps, w["wfl"][:, cb, ob, :], igft[:, cb, :], cb == 0, cb == 1)
                    nc.scalar.activation(src_sb[:, ob, :], ps, AF.Sigmoid,
                                         bias=w["bhd"][:, ob:ob + 1])

                fo = hp.tile([96, N], f32r, tag="fo")
                ps = pspt.tile([96, N], f32, tag="ps_pt")
                for cb in range(2):
                    mmr(ps, w["wfrl"][:, cb, :], src_sb[:, cb, :], cb == 0, cb == 1)
                nc.vector.tensor_scalar_add(fo, ps, w["bhd"][:, 2:3])

                h1 = hp.tile([96, N], f32r, tag="h1")
                ps = pspt.tile([96, N], f32, tag="ps_pt")
                mmr(ps, w["wfc1"][:, :], fo[:, :], True, True)
                nc.scalar.activation(h1, ps, AF.Lrelu, bias=w["bhd"][:, 3:4],
                                     alpha=0.01)

                osb = hp.tile([1, N], f32, tag="osb")
                ps2 = pspt.tile([1, N], f32, tag="ps_pt")
                mmr(ps2, w["wfc2"][:, :], h1[:, :], True, True)
                nc.vector.tensor_scalar_add(osb, ps2, w["bfc2"][0:1, 0:1])
                nc.sync.dma_start(out=d_out[b:b + 1, :], in_=osb)

    nc.compile()
    return nc


# ----------------------------------------------------------------------------
# entry point
# ----------------------------------------------------------------------------

def kernel(**inputs):
    global LAST_RESULTS
    from concourse import bass_utils
    import concourse.mybir as mybir

    inputs = {k: np.asarray(v) for k, v in inputs.items()}
    dev, att = _host_precompute(inputs)
    x_shards = _pack_x(inputs["x"].astype(np.float32))

    f32 = mybir.dt.float32
    f32r = mybir.dt.float32r
    bf16 = mybir.dt.bfloat16
    head_r = {"wms", "wfl", "wfrl", "wfc1", "wfc2"}
    dev_shapes = [(k, v.shape,
                   bf16 if v.dtype == BF16 else (f32r if k in head_r else f32))
                  for k, v in dev.items()]

    key = tuple((n, tuple(s), str(dt_)) for n, s, dt_ in dev_shapes)
    if key not in _CACHE:
        _CACHE[key] = _build_program(dev_shapes)
    nc = _CACHE[key]

    in_maps = []
    for c in range(NCORES):
        m = dict(dev)
        m["xt"] = np.ascontiguousarray(x_shards[c])
        in_maps.append(m)

    trace = bool(int(os.environ.get("KERNEL_TRACE", "0")))
    res = bass_utils.run_bass_kernel_spmd(
        nc, in_maps, core_ids=list(range(NCORES)), trace=trace)
    LAST_RESULTS = res

    out = np.concatenate([r["out"] for r in res.results], axis=0)  # (16, 512)
    forecast = out.reshape(B, 1, N).astype(np.float32)
    return forecast, att


# revision 8
# speedup vs baseline: 1.3242x; 1.3242x over previous
"""Trainium2 Bass kernel for nn_Model_17291538333963 (gnn_message_passing).

Structure of the model (B=16, T=96, N=H=512, MT=192, C4=768):
  - The GRU runs on x*0 (zeros!), so the whole GRU -> attention -> Laplacian ->
    Chebyshev chain depends ONLY on weights and is identical across batch.
    It is computed once on host (pure weight preprocessing).
  - The length-4 FFT (with cheb[0] == 0) and irfft collapse to small linear
    combinations which are folded into the surrounding weight matrices.
  - The remaining work is a dense, batch-parallel pipeline:
       P = combos(L) @ Xp        (4 streams a,c,e,d)
       GLU-A (288/96 -> 768), GLU-B1 (768 -> 768), GLU-B2 (768 -> 576/192)
       head: igft (fold irfft+weight_param), fl+sigmoid, frl, fc1+leaky, fc2
    which is sharded data-parallel over batch: 2 batch items per core x 8.
  - All activations flow through the device in TRANSPOSED layout
    (features on partitions, nodes on the free dim), so every matmul is
    out[p,f] = sum_c lhsT[c,p] rhs[c,f] with host-pretransposed weights.
"""

import os
import numpy as np
import ml_dtypes

B, T, N = 16, 96, 512
MT = 192
C4 = 768
NCORES = 8
BPC = B // NCORES  # 2 batch items per core

BF16 = ml_dtypes.bfloat16

# cached compiled program + dram tensor handles
_CACHE = {}
LAST_RESULTS = None  # BassKernelResults of the most recent run (for profiling)


# ----------------------------------------------------------------------------
# host-side precompute (weight-only math + layout packing)
# ----------------------------------------------------------------------------

def _sigmoid(x):
    out = np.empty_like(x)
    np.negative(x, out=out)
    np.exp(out, out=out)
    out += 1.0
    np.reciprocal(out, out=out)
    return out


def _host_precompute(inp):
    """All math that depends only on weights; returns packed device tensors
    (shared across cores) plus the replicated attention matrix output."""
    H = N
    f8 = np.float64
    # --- GRU over 512 steps with zero input, single row (batch-identical) ---
    b_ih = inp["b_ih"].astype(f8)
    W_hh = inp["W_hh"].astype(f8)
    b_hh = inp["b_hh"].astype(f8)
    xr, xz, xn = b_ih[:H], b_ih[H:2 * H], b_ih[2 * H:]
    h = np.zeros(H, f8)
    outs = np.empty((N, H), f8)
    for t in range(N):
        gh = W_hh @ h + b_hh
        r = _sigmoid(xr + gh[:H])
        z = _sigmoid(xz + gh[H:2 * H])
        nn_ = np.tanh(xn + r * gh[2 * H:])
        h = (1.0 - z) * nn_ + z * h
        outs[t] = h
    # --- attention (identical for every batch element) ---
    key = outs.T @ inp["wk"].astype(f8)[:, 0]
    qry = outs.T @ inp["wq"].astype(f8)[:, 0]
    e = key[:, None] + qry[None, :]
    e = np.where(e > 0, e, 0.2 * e)
    e -= e.max(axis=1, keepdims=True)
    att = np.exp(e)
    att /= att.sum(axis=1, keepdims=True)
    # --- laplacian + chebyshev combos ---
    deg = att.sum(axis=1)
    att_sym = 0.5 * (att + att.T)
    d_inv = 1.0 / (np.sqrt(deg) + 1e-7)
    lap = d_inv[:, None] * (np.diag(deg) - att_sym) * d_inv[None, :]
    L2 = lap
    L3 = 2.0 * lap @ L2
    L4 = 2.0 * lap @ L3 - L2
    # gft streams: a = g1+g2+g3, c = -g2, e = -g1+g2-g3, d = g3-g1
    Lmats = np.stack([L2 + L3 + L4, -L3, -L2 + L3 - L4, L4 - L2]).astype(np.float32)

    dev = {}
    # lmT[p, j, c, n] = Lmats[j].T[c*128+p, n]
    lmT = np.ascontiguousarray(
        Lmats.transpose(0, 2, 1).reshape(4, 4, 128, N).transpose(2, 0, 1, 3))
    dev["lmT"] = lmT.astype(BF16)

    # --- GLU-A folded weights ---
    def fold_real(W):  # (C4, 4T) -> (C4, 3T): [W0, W1+W3, W2]
        W0, W1, W2, W3 = W[:, :T], W[:, T:2 * T], W[:, 2 * T:3 * T], W[:, 3 * T:]
        return np.concatenate([W0, W1 + W3, W2], axis=1)

    def fold_imag(W):  # imag input = [0, d, 0, -d] -> W1 - W3
        return W[:, T:2 * T] - W[:, 3 * T:]

    # wa_re[p, proj, ci, o] = W_eff[o, ci*96+p]
    wa_re = np.stack([fold_real(inp["gluA_lW"][0]), fold_real(inp["gluA_rW"][0])])
    dev["wa_re"] = np.ascontiguousarray(
        wa_re.transpose(2, 0, 1).reshape(3, 96, 2, C4).transpose(1, 2, 0, 3)
    ).astype(BF16)
    wa_im = np.stack([fold_imag(inp["gluA_lW"][1]), fold_imag(inp["gluA_rW"][1])])
    dev["wa_im"] = np.ascontiguousarray(wa_im.transpose(2, 0, 1)).astype(BF16)

    # wb1[p, mi, ci, o] = Wmi[o, ci*128+p]
    wb1 = np.stack([inp["gluB_lW"][0], inp["gluB_rW"][0],
                    inp["gluB_lW"][1], inp["gluB_rW"][1]])  # (4, 768, 768)
    dev["wb1"] = np.ascontiguousarray(
        wb1.transpose(2, 0, 1).reshape(6, 128, 4, C4).transpose(1, 2, 0, 3)
    ).astype(BF16)

    wb2r = np.stack([inp["gluB_lW"][2][:576], inp["gluB_rW"][2][:576]])  # (2,576,768)
    dev["wb2r"] = np.ascontiguousarray(
        wb2r.transpose(2, 0, 1).reshape(6, 128, 2, 576).transpose(1, 2, 0, 3)
    ).astype(BF16)
    wb2i = np.stack([inp["gluB_lW"][3][192:384], inp["gluB_rW"][3][192:384]])
    dev["wb2i"] = np.ascontiguousarray(
        wb2i.transpose(2, 0, 1).reshape(6, 128, 2, MT).transpose(1, 2, 0, 3)
    ).astype(BF16)

    # --- head: fold irfft + weight_param ---
    Wp = inp["weight_param"].astype(f8)
    Ms = np.stack([(Wp[0] + Wp[1] + Wp[2] + Wp[3]) / 4.0,
                   (Wp[0] - Wp[2]) / 2.0,
                   (Wp[0] - Wp[1] + Wp[2] - Wp[3]) / 4.0,
                   (Wp[3] - Wp[1]) / 2.0]).astype(np.float32)  # (4, 192, 192)
    # wms[c, j, tb, ub, u] = Ms[j][tb*96+c, ub*96+u]
    dev["wms"] = np.ascontiguousarray(
        Ms.reshape(4, 2, 96, 2, 96).transpose(2, 0, 1, 3, 4))
    # wfl[c, cb, ob, u] = fl_W[ob*96+u, cb*96+c]
    flW = inp["fl_W"].astype(np.float32)  # (192, 192)
    dev["wfl"] = np.ascontiguousarray(
        flW.reshape(2, 96, 2, 96).transpose(3, 2, 0, 1))
    # wfrl[c, cb, u] = frl_W[u, cb*96+c]
    dev["wfrl"] = np.ascontiguousarray(
        inp["frl_W"].astype(np.float32).reshape(T, 2, 96).transpose(2, 1, 0))
    dev["wfc1"] = np.ascontiguousarray(inp["fc1_W"].astype(np.float32).T)  # (96, 96)
    dev["wfc2"] = np.ascontiguousarray(inp["fc2_W"].astype(np.float32).T)  # (96, 1)

    # --- biases ---
    ba = np.stack([np.stack([inp["gluA_lb"][0], inp["gluA_rb"][0]]),
                   np.stack([inp["gluA_lb"][1], inp["gluA_rb"][1]])])  # (2,2,768)
    dev["ba"] = np.ascontiguousarray(
        ba.reshape(2, 2, 6, 128).transpose(3, 0, 1, 2)).astype(np.float32)
    bb1 = np.stack([inp["gluB_lb"][0], inp["gluB_rb"][0],
                    inp["gluB_lb"][1], inp["gluB_rb"][1]])  # (4, 768)
    dev["bb1"] = np.ascontiguousarray(
        bb1.reshape(4, 6, 128).transpose(2, 0, 1)).astype(np.float32)
    # bb2[p, proj, oc]: oc 0..5 real chunks of 96, oc 6..7 imag chunks
    bb2 = np.empty((96, 2, 8), np.float32)
    for pi, (br, bi) in enumerate([(inp["gluB_lb"][2], inp["gluB_lb"][3]),
                                   (inp["gluB_rb"][2], inp["gluB_rb"][3])]):
        bb2[:, pi, :6] = np.asarray(br[:576]).reshape(6, 96).T
        bb2[:, pi, 6:] = np.asarray(bi[192:384]).reshape(2, 96).T
    dev["bb2"] = bb2
    bhd = np.empty((96, 4), np.float32)
    bhd[:, 0:2] = np.asarray(inp["fl_b"]).reshape(2, 96).T
    bhd[:, 2] = np.asarray(inp["frl_b"])
    bhd[:, 3] = np.asarray(inp["fc1_b"])
    dev["bhd"] = bhd
    dev["bfc2"] = np.asarray(inp["fc2_b"]).astype(np.float32).reshape(1, 1)

    return dev, att.astype(np.float32)


def _pack_x(x):
    """x (B, T, N) -> per-core shards (BPC, 128, 4, T) bf16 with
    [b, p, c, t] = x[b][t, c*128+p]."""
    xt = np.ascontiguousarray(
        x.transpose(0, 2, 1).reshape(B, 4, 128, T).transpose(0, 2, 1, 3))
    xt = xt.astype(BF16)
    return [xt[i * BPC:(i + 1) * BPC] for i in range(NCORES)]


# ----------------------------------------------------------------------------
# device program
# ----------------------------------------------------------------------------

def _build_program(dev_shapes):
    import concourse.bass as bass  # noqa: F401
    import concourse.mybir as mybir
    import concourse.tile as tile
    from concourse import bacc

    f32 = mybir.dt.float32
    f32r = mybir.dt.float32r
    bf16 = mybir.dt.bfloat16
    AF = mybir.ActivationFunctionType
    OP = mybir.AluOpType

    nc = bacc.Bacc("TRN2", target_bir_lowering=False)

    d = {}
    d["xt"] = nc.dram_tensor("xt", (BPC, 128, 4, T), bf16, kind="ExternalInput")
    for name, arr_shape, dt_ in dev_shapes:
        d[name] = nc.dram_tensor(name, arr_shape, dt_, kind="ExternalInput")
    d_out = nc.dram_tensor("out", (BPC, N), f32, kind="ExternalOutput")

    with tile.TileContext(nc) as tc:
        with (
            tc.tile_pool(name="wp", bufs=1) as wp,
            tc.tile_pool(name="xpool", bufs=2) as xpool,
            tc.tile_pool(name="ptp", bufs=2) as ptp,
            tc.tile_pool(name="apool", bufs=2) as apool,
            tc.tile_pool(name="bpool", bufs=2) as bpool,
            tc.tile_pool(name="rpool", bufs=1) as rpool,
            tc.tile_pool(name="hp", bufs=1) as hp,
            tc.tile_pool(name="sgp", bufs=4) as sgp,
            tc.tile_pool(name="pspt", bufs=2, space="PSUM") as pspt,
            tc.tile_pool(name="psglu", bufs=3, space="PSUM") as psglu,
        ):
            # ---- resident weights ----
            w = {}
            wt_specs = [
                # ordered by consumption time: small biases + first-phase
                # weights first so compute starts while big weights stream in
                ("ba", [128, 2, 2, 6], f32),
                ("bb1", [128, 4, 6], f32),
                ("bb2", [96, 2, 8], f32),
                ("bhd", [96, 4], f32),
                ("bfc2", [1, 1], f32),
                ("lmT", [128, 4, 4, N], bf16),
                ("wa_re", [96, 2, 3, C4], bf16),
                ("wa_im", [96, 2, C4], bf16),
                ("wb1", [128, 4, 6, C4], bf16),
                ("wb2r", [128, 2, 6, 576], bf16),
                ("wb2i", [128, 2, 6, MT], bf16),
                ("wms", [96, 4, 2, 2, 96], f32r),
                ("wfl", [96, 2, 2, 96], f32r),
                ("wfrl", [96, 2, 96], f32r),
                ("wfc1", [96, 96], f32r),
                ("wfc2", [96, 1], f32r),
            ]
            xp_tiles = []
            for b in range(BPC):
                xp = xpool.tile([128, 4, T], bf16, tag="xp", name=f"xp{b}")
                nc.sync.dma_start(out=xp, in_=d["xt"][b])
                xp_tiles.append(xp)
            for name, shp, dt_ in wt_specs:
                w[name] = wp.tile(shp, dt_, tag=name, name=name)
                nc.sync.dma_start(out=w[name], in_=d[name][:])

            for b in range(BPC):
                # ---- chebyshev streams: P^T[j] (96 x 512), j in {a,c,e,d} ----
                xp = xp_tiles[b]
                pts = ptp.tile([96, 4, N], bf16, tag="pts")
                for j in range(4):
                    ps = pspt.tile([96, N], f32, tag="ps_pt")
                    for c in range(4):
                        nc.tensor.matmul(ps, lhsT=xp[:, c, :], rhs=w["lmT"][:, j, c, :],
                                         start=(c == 0), stop=(c == 3))
                    nc.vector.tensor_copy(pts[:, j, :], ps)

                # ---- GLU A ----
                realA = apool.tile([128, 6, N], bf16, tag="realA")
                imagA = apool.tile([128, 6, N], bf16, tag="imagA")
                for path in range(2):
                    ncis = 3 if path == 0 else 1
                    dst = realA if path == 0 else imagA
                    for oi in range(6):
                        psl = psglu.tile([128, N], f32, tag="psl")
                        psr = psglu.tile([128, N], f32, tag="psr")
                        for ci in range(ncis):
                            rhs = pts[:, ci, :] if path == 0 else pts[:, 3, :]
                            if path == 0:
                                ll = w["wa_re"][:, 0, ci, oi * 128:(oi + 1) * 128]
                                lr = w["wa_re"][:, 1, ci, oi * 128:(oi + 1) * 128]
                            else:
                                ll = w["wa_im"][:, 0, oi * 128:(oi + 1) * 128]
                                lr = w["wa_im"][:, 1, oi * 128:(oi + 1) * 128]
                            nc.tensor.matmul(psl, lhsT=ll, rhs=rhs,
                                             start=(ci == 0), stop=(ci == ncis - 1))
                            nc.tensor.matmul(psr, lhsT=lr, rhs=rhs,
                                             start=(ci == 0), stop=(ci == ncis - 1))
                        sig = sgp.tile([128, N], f32, tag="sig")
                        nc.scalar.activation(sig, psr, AF.Sigmoid,
                                             bias=w["ba"][:, path, 1, oi:oi + 1])
                        nc.vector.scalar_tensor_tensor(
                            out=dst[:, oi, :], in0=psl,
                            scalar=w["ba"][:, path, 0, oi:oi + 1], in1=sig,
                            op0=OP.add, op1=OP.mult)

                # ---- GLU B1 ----
                realB = bpool.tile([128, 6, N], bf16, tag="realB")
                imagB = bpool.tile([128, 6, N], bf16, tag="imagB")
                for path in range(2):
                    src = realA if path == 0 else imagA
                    dst = realB if path == 0 else imagB
                    ml, mr = (0, 1) if path == 0 else (2, 3)
                    for oi in range(6):
                        psl = psglu.tile([128, N], f32, tag="psl")
                        psr = psglu.tile([128, N], f32, tag="psr")
                        for ci in range(6):
                            rhs = src[:, ci, :]
                            nc.tensor.matmul(
                                psl, lhsT=w["wb1"][:, ml, ci, oi * 128:(oi + 1) * 128],
                                rhs=rhs, start=(ci == 0), stop=(ci == 5))
                            nc.tensor.matmul(
                                psr, lhsT=w["wb1"][:, mr, ci, oi * 128:(oi + 1) * 128],
                                rhs=rhs, start=(ci == 0), stop=(ci == 5))
                        sig = sgp.tile([128, N], f32, tag="sig")
                        nc.scalar.activation(sig, psr, AF.Sigmoid,
                                             bias=w["bb1"][:, mr, oi:oi + 1])
                        nc.vector.scalar_tensor_tensor(
                            out=dst[:, oi, :], in0=psl,
                            scalar=w["bb1"][:, ml, oi:oi + 1], in1=sig,
                            op0=OP.add, op1=OP.mult)

                # ---- GLU B2 (only the output chunks the head consumes) ----
                Rsb = rpool.tile([96, 6, N], f32r, tag="Rsb")
                Isb = rpool.tile([96, 2, N], f32r, tag="Isb")
                for path in range(2):
                    noc = 6 if path == 0 else 2
                    src = realB if path == 0 else imagB
                    wgt = w["wb2r"] if path == 0 else w["wb2i"]
                    dst = Rsb if path == 0 else Isb
                    for oc in range(noc):
                        psl = psglu.tile([96, N], f32, tag="psl")
                        psr = psglu.tile([96, N], f32, tag="psr")
                        for ci in range(6):
                            rhs = src[:, ci, :]
                            nc.tensor.matmul(
                                psl, lhsT=wgt[:, 0, ci, oc * 96:(oc + 1) * 96],
                                rhs=rhs, start=(ci == 0), stop=(ci == 5))
                            nc.tensor.matmul(
                                psr, lhsT=wgt[:, 1, ci, oc * 96:(oc + 1) * 96],
                                rhs=rhs, start=(ci == 0), stop=(ci == 5))
                        boff = oc if path == 0 else 6 + oc
                        sig = sgp.tile([96, N], f32, tag="sig")
                        nc.scalar.activation(sig, psr, AF.Sigmoid,
                                             bias=w["bb2"][:, 1, boff:boff + 1])
                        nc.vector.scalar_tensor_tensor(
                            out=dst[:, oc, :], in0=psl,
                            scalar=w["bb2"][:, 0, boff:boff + 1], in1=sig,
                            op0=OP.add, op1=OP.mult)

                # ---- head (float32r matmuls) ----
                def mmr(ps, lhsT, rhs, start, stop):
                    nc.tensor.matmul(ps, lhsT=lhsT, rhs=rhs, start=start, stop=stop)

                igft = hp.tile([96, 2, N], f32r, tag="igft")
                for ub in range(2):
                    ps = pspt.tile([96, N], f32, tag="ps_pt")
                    k = 0
                    for j in range(3):
                        for tb in range(2):
                            mmr(ps, w["wms"][:, j, tb, ub, :], Rsb[:, 2 * j + tb, :],
                                k == 0, k == 7)
                            k += 1
                    for tb in range(2):
                        mmr(ps, w["wms"][:, 3, tb, ub, :], Isb[:, tb, :], k == 0, k == 7)
                        k += 1
                    nc.scalar.copy(igft[:, ub, :], ps)

                src_sb = hp.tile([96, 2, N], f32r, tag="srcsb")
                for ob in range(2):
                    ps = pspt.tile([96, N], f32, tag="ps_pt")
                    for cb in range(2):
                        mmr(ps, w["wfl"][:, cb, ob, :], igft[:, cb, :], cb == 0, cb == 1)
                    nc.scalar.activation(src_sb[:, ob, :], ps, AF.Sigmoid,
                                         bias=w["bhd"][:, ob:ob + 1])

                fo = hp.tile([96, N], f32r, tag="fo")
                ps = pspt.tile([96, N], f32, tag="ps_pt")
                for cb in range(2):
                    mmr(ps, w["wfrl"][:, cb, :], src_sb[:, cb, :], cb == 0, cb == 1)
                nc.vector.tensor_scalar_add(fo, ps, w["bhd"][:, 2:3])

                h1 = hp.tile([96, N], f32r, tag="h1")
                ps = pspt.tile([96, N], f32, tag="ps_pt")
                mmr(ps, w["wfc1"][:, :], fo[:, :], True, True)
                nc.scalar.activation(h1, ps, AF.Lrelu, bias=w["bhd"][:, 3:4],
                                     alpha=0.01)

                osb = hp.tile([1, N], f32, tag="osb")
                ps2 = pspt.tile([1, N], f32, tag="ps_pt")
                mmr(ps2, w["wfc2"][:, :], h1[:, :], True, True)
                nc.vector.tensor_scalar_add(osb, ps2, w["bfc2"][0:1, 0:1])
                nc.sync.dma_start(out=d_out[b:b + 1, :], in_=osb)

    nc.compile()
    return nc


# ----------------------------------------------------------------------------
# entry point
# ----------------------------------------------------------------------------

def kernel(**inputs):
    global LAST_RESULTS
    from concourse import bass_utils
    import concourse.mybir as mybir

    inputs = {k: np.asarray(v) for k, v in inputs.items()}
    dev, att = _host_precompute(inputs)
    x_shards = _pack_x(inputs["x"].astype(np.float32))

    f32 = mybir.dt.float32
    f32r = mybir.dt.float32r
    bf16 = mybir.dt.bfloat16
    head_r = {"wms", "wfl", "wfrl", "wfc1", "wfc2"}
    dev_shapes = [(k, v.shape,
                   bf16 if v.dtype == BF16 else (f32r if k in head_r else f32))
                  for k, v in dev.items()]

    key = tuple((n, tuple(s), str(dt_)) for n, s, dt_ in dev_shapes)
    if key not in _CACHE:
        _CACHE[key] = _build_program(dev_shapes)
    nc = _CACHE[key]

    in_maps = []
    for c in range(NCORES):
        m = dict(dev)
        m["xt"] = np.ascontiguousarray(x_shards[c])
        in_maps.append(m)

    trace = bool(int(os.environ.get("KERNEL_TRACE", "0")))
    res = bass_utils.run_bass_kernel_spmd(
        nc, in_maps, core_ids=list(range(NCORES)), trace=trace)
    LAST_RESULTS = res

    out = np.concatenate([r["out"] for r in res.results], axis=0)  # (16, 512)
    forecast = out.reshape(B, 1, N).astype(np.float32)
    return forecast, att


# revision 9
# speedup vs baseline: 1.3496x; 1.0192x over previous
"""Trainium2 Bass kernel for nn_Model_17291538333963 (gnn_message_passing).

Structure of the model (B=16, T=96, N=H=512, MT=192, C4=768):
  - The GRU runs on x*0 (zeros!), so the whole GRU -> attention -> Laplacian ->
    Chebyshev chain depends ONLY on weights and is identical across batch.
    It is computed once on host (pure weight preprocessing).
  - The length-4 FFT (with cheb[0] == 0) and irfft collapse to small linear
    combinations which are folded into the surrounding weight matrices.
  - The remaining work is a dense, batch-parallel pipeline:
       P = combos(L) @ Xp        (4 streams a,c,e,d)
       GLU-A (288/96 -> 768), GLU-B1 (768 -> 768), GLU-B2 (768 -> 576/192)
       head: igft (fold irfft+weight_param), fl+sigmoid, frl, fc1+leaky, fc2
    which is sharded data-parallel over batch: 2 batch items per core x 8.
  - All activations flow through the device in TRANSPOSED layout
    (features on partitions, nodes on the free dim), so every matmul is
    out[p,f] = sum_c lhsT[c,p] rhs[c,f] with host-pretransposed weights.
"""

import os
import numpy as np
import ml_dtypes

B, T, N = 16, 96, 512
MT = 192
C4 = 768
NCORES = 8
BPC = B // NCORES  # 2 batch items per core

BF16 = ml_dtypes.bfloat16

# cached compiled program + dram tensor handles
_CACHE = {}
LAST_RESULTS = None  # BassKernelResults of the most recent run (for profiling)


# ----------------------------------------------------------------------------
# host-side precompute (weight-only math + layout packing)
# ----------------------------------------------------------------------------

def _sigmoid(x):
    out = np.empty_like(x)
    np.negative(x, out=out)
    np.exp(out, out=out)
    out += 1.0
    np.reciprocal(out, out=out)
    return out


def _host_precompute(inp):
    """All math that depends only on weights; returns packed device tensors
    (shared across cores) plus the replicated attention matrix output."""
    H = N
    f8 = np.float64
    # --- GRU over 512 steps with zero input, single row (batch-identical) ---
    b_ih = inp["b_ih"].astype(f8)
    W_hh = inp["W_hh"].astype(f8)
    b_hh = inp["b_hh"].astype(f8)
    xr, xz, xn = b_ih[:H], b_ih[H:2 * H], b_ih[2 * H:]
    h = np.zeros(H, f8)
    outs = np.empty((N, H), f8)
    for t in range(N):
        gh = W_hh @ h + b_hh
        r = _sigmoid(xr + gh[:H])
        z = _sigmoid(xz + gh[H:2 * H])
        nn_ = np.tanh(xn + r * gh[2 * H:])
        h = (1.0 - z) * nn_ + z * h
        outs[t] = h
    # --- attention (identical for every batch element) ---
    key = outs.T @ inp["wk"].astype(f8)[:, 0]
    qry = outs.T @ inp["wq"].astype(f8)[:, 0]
    e = key[:, None] + qry[None, :]
    e = np.where(e > 0, e, 0.2 * e)
    e -= e.max(axis=1, keepdims=True)
    att = np.exp(e)
    att /= att.sum(axis=1, keepdims=True)
    # --- laplacian + chebyshev combos ---
    deg = att.sum(axis=1)
    att_sym = 0.5 * (att + att.T)
    d_inv = 1.0 / (np.sqrt(deg) + 1e-7)
    lap = d_inv[:, None] * (np.diag(deg) - att_sym) * d_inv[None, :]
    L2 = lap
    L3 = 2.0 * lap @ L2
    L4 = 2.0 * lap @ L3 - L2
    # gft streams: a = g1+g2+g3, c = -g2, e = -g1+g2-g3, d = g3-g1
    Lmats = np.stack([L2 + L3 + L4, -L3, -L2 + L3 - L4, L4 - L2]).astype(np.float32)

    dev = {}
    # lmT[p, j, c, n] = Lmats[j].T[c*128+p, n]
    lmT = np.ascontiguousarray(
        Lmats.transpose(0, 2, 1).reshape(4, 4, 128, N).transpose(2, 0, 1, 3))
    dev["lmT"] = lmT.astype(BF16)

    # --- GLU-A folded weights ---
    def fold_real(W):  # (C4, 4T) -> (C4, 3T): [W0, W1+W3, W2]
        W0, W1, W2, W3 = W[:, :T], W[:, T:2 * T], W[:, 2 * T:3 * T], W[:, 3 * T:]
        return np.concatenate([W0, W1 + W3, W2], axis=1)

    def fold_imag(W):  # imag input = [0, d, 0, -d] -> W1 - W3
        return W[:, T:2 * T] - W[:, 3 * T:]

    # wa_re[p, proj, ci, o] = W_eff[o, ci*96+p]
    wa_re = np.stack([fold_real(inp["gluA_lW"][0]), fold_real(inp["gluA_rW"][0])])
    dev["wa_re"] = np.ascontiguousarray(
        wa_re.transpose(2, 0, 1).reshape(3, 96, 2, C4).transpose(1, 2, 0, 3)
    ).astype(BF16)
    wa_im = np.stack([fold_imag(inp["gluA_lW"][1]), fold_imag(inp["gluA_rW"][1])])
    dev["wa_im"] = np.ascontiguousarray(wa_im.transpose(2, 0, 1)).astype(BF16)

    # wb1[p, mi, ci, o] = Wmi[o, ci*128+p]
    wb1 = np.stack([inp["gluB_lW"][0], inp["gluB_rW"][0],
                    inp["gluB_lW"][1], inp["gluB_rW"][1]])  # (4, 768, 768)
    dev["wb1"] = np.ascontiguousarray(
        wb1.transpose(2, 0, 1).reshape(6, 128, 4, C4).transpose(1, 2, 0, 3)
    ).astype(BF16)

    wb2r = np.stack([inp["gluB_lW"][2][:576], inp["gluB_rW"][2][:576]])  # (2,576,768)
    dev["wb2r"] = np.ascontiguousarray(
        wb2r.transpose(2, 0, 1).reshape(6, 128, 2, 576).transpose(1, 2, 0, 3)
    ).astype(BF16)
    wb2i = np.stack([inp["gluB_lW"][3][192:384], inp["gluB_rW"][3][192:384]])
    dev["wb2i"] = np.ascontiguousarray(
        wb2i.transpose(2, 0, 1).reshape(6, 128, 2, MT).transpose(1, 2, 0, 3)
    ).astype(BF16)

    # --- head: fold irfft + weight_param ---
    Wp = inp["weight_param"].astype(f8)
    Ms = np.stack([(Wp[0] + Wp[1] + Wp[2] + Wp[3]) / 4.0,
                   (Wp[0] - Wp[2]) / 2.0,
                   (Wp[0] - Wp[1] + Wp[2] - Wp[3]) / 4.0,
                   (Wp[3] - Wp[1]) / 2.0]).astype(np.float32)  # (4, 192, 192)
    # wms[c, j, tb, ub, u] = Ms[j][tb*96+c, ub*96+u]
    dev["wms"] = np.ascontiguousarray(
        Ms.reshape(4, 2, 96, 2, 96).transpose(2, 0, 1, 3, 4))
    # wfl[c, cb, ob, u] = fl_W[ob*96+u, cb*96+c]
    flW = inp["fl_W"].astype(np.float32)  # (192, 192)
    dev["wfl"] = np.ascontiguousarray(
        flW.reshape(2, 96, 2, 96).transpose(3, 2, 0, 1))
    # wfrl[c, cb, u] = frl_W[u, cb*96+c]
    dev["wfrl"] = np.ascontiguousarray(
        inp["frl_W"].astype(np.float32).reshape(T, 2, 96).transpose(2, 1, 0))
    dev["wfc1"] = np.ascontiguousarray(inp["fc1_W"].astype(np.float32).T)  # (96, 96)
    dev["wfc2"] = np.ascontiguousarray(inp["fc2_W"].astype(np.float32).T)  # (96, 1)

    # --- biases ---
    ba = np.stack([np.stack([inp["gluA_lb"][0], inp["gluA_rb"][0]]),
                   np.stack([inp["gluA_lb"][1], inp["gluA_rb"][1]])])  # (2,2,768)
    dev["ba"] = np.ascontiguousarray(
        ba.reshape(2, 2, 6, 128).transpose(3, 0, 1, 2)).astype(np.float32)
    bb1 = np.stack([inp["gluB_lb"][0], inp["gluB_rb"][0],
                    inp["gluB_lb"][1], inp["gluB_rb"][1]])  # (4, 768)
    dev["bb1"] = np.ascontiguousarray(
        bb1.reshape(4, 6, 128).transpose(2, 0, 1)).astype(np.float32)
    # bb2[p, proj, oc]: oc 0..5 real chunks of 96, oc 6..7 imag chunks
    bb2 = np.empty((96, 2, 8), np.float32)
    for pi, (br, bi) in enumerate([(inp["gluB_lb"][2], inp["gluB_lb"][3]),
                                   (inp["gluB_rb"][2], inp["gluB_rb"][3])]):
        bb2[:, pi, :6] = np.asarray(br[:576]).reshape(6, 96).T
        bb2[:, pi, 6:] = np.asarray(bi[192:384]).reshape(2, 96).T
    dev["bb2"] = bb2
    bhd = np.empty((96, 4), np.float32)
    bhd[:, 0:2] = np.asarray(inp["fl_b"]).reshape(2, 96).T
    bhd[:, 2] = np.asarray(inp["frl_b"])
    bhd[:, 3] = np.asarray(inp["fc1_b"])
    dev["bhd"] = bhd
    dev["bfc2"] = np.asarray(inp["fc2_b"]).astype(np.float32).reshape(1, 1)

    return dev, att.astype(np.float32)


def _pack_x(x):
    """x (B, T, N) -> per-core shards (BPC, 128, 4, T) bf16 with
    [b, p, c, t] = x[b][t, c*128+p]."""
    xt = np.ascontiguousarray(
        x.transpose(0, 2, 1).reshape(B, 4, 128, T).transpose(0, 2, 1, 3))
    xt = xt.astype(BF16)
    return [xt[i * BPC:(i + 1) * BPC] for i in range(NCORES)]


# ----------------------------------------------------------------------------
# device program
# ----------------------------------------------------------------------------

def _build_program(dev_shapes):
    import concourse.bass as bass  # noqa: F401
    import concourse.mybir as mybir
    import concourse.tile as tile
    from concourse import bacc

    f32 = mybir.dt.float32
    f32r = mybir.dt.float32r
    bf16 = mybir.dt.bfloat16
    AF = mybir.ActivationFunctionType
    OP = mybir.AluOpType

    nc = bacc.Bacc("TRN2", target_bir_lowering=False)

    d = {}
    d["xt"] = nc.dram_tensor("xt", (BPC, 128, 4, T), bf16, kind="ExternalInput")
    for name, arr_shape, dt_ in dev_shapes:
        d[name] = nc.dram_tensor(name, arr_shape, dt_, kind="ExternalInput")
    d_out = nc.dram_tensor("out", (BPC, N), f32, kind="ExternalOutput")

    with tile.TileContext(nc) as tc:
        with (
            tc.tile_pool(name="wp", bufs=1) as wp,
            tc.tile_pool(name="xpool", bufs=2) as xpool,
            tc.tile_pool(name="ptp", bufs=2) as ptp,
            tc.tile_pool(name="apool", bufs=2) as apool,
            tc.tile_pool(name="bpool", bufs=2) as bpool,
            tc.tile_pool(name="rpool", bufs=1) as rpool,
            tc.tile_pool(name="hp", bufs=1) as hp,
            tc.tile_pool(name="sgp", bufs=4) as sgp,
            tc.tile_pool(name="pspt", bufs=2, space="PSUM") as pspt,
            tc.tile_pool(name="psglu", bufs=6, space="PSUM") as psglu,
        ):
            # ---- resident weights ----
            w = {}
            wt_specs = [
                # ordered by consumption time so compute starts while the
                # big weights are still streaming in
                ("lmT", [128, 4, 4, N], bf16),
                ("wa_re", [96, 2, 3, C4], bf16),
                ("wa_im", [96, 2, C4], bf16),
                ("ba", [128, 2, 2, 6], f32),
                ("bb1", [128, 4, 6], f32),
                ("bb2", [96, 2, 8], f32),
                ("bhd", [96, 4], f32),
                ("bfc2", [1, 1], f32),
                ("wb1", [128, 4, 6, C4], bf16),
                ("wb2r", [128, 2, 6, 576], bf16),
                ("wb2i", [128, 2, 6, MT], bf16),
                ("wms", [96, 4, 2, 2, 96], f32r),
                ("wfl", [96, 2, 2, 96], f32r),
                ("wfrl", [96, 2, 96], f32r),
                ("wfc1", [96, 96], f32r),
                ("wfc2", [96, 1], f32r),
            ]
            xp_tiles = []
            for b in range(BPC):
                xp = xpool.tile([128, 4, T], bf16, tag="xp", name=f"xp{b}")
                nc.sync.dma_start(out=xp, in_=d["xt"][b])
                xp_tiles.append(xp)
            for name, shp, dt_ in wt_specs:
                w[name] = wp.tile(shp, dt_, tag=name, name=name)
                if name == "lmT":
                    for j in range(4):  # per-j so cheb j=0 starts early
                        nc.sync.dma_start(out=w[name][:, j], in_=d[name][:, j])
                else:
                    nc.sync.dma_start(out=w[name], in_=d[name][:])

            for b in range(BPC):
                # ---- chebyshev streams: P^T[j] (96 x 512), j in {a,c,e,d} ----
                xp = xp_tiles[b]
                pts = ptp.tile([96, 4, N], bf16, tag="pts")
                for j in range(4):
                    ps = pspt.tile([96, N], f32, tag="ps_pt")
                    for c in range(4):
                        nc.tensor.matmul(ps, lhsT=xp[:, c, :], rhs=w["lmT"][:, j, c, :],
                                         start=(c == 0), stop=(c == 3))
                    nc.vector.tensor_copy(pts[:, j, :], ps)

                # ---- GLU A ----
                realA = apool.tile([128, 6, N], bf16, tag="realA")
                imagA = apool.tile([128, 6, N], bf16, tag="imagA")
                for path in range(2):
                    ncis = 3 if path == 0 else 1
                    dst = realA if path == 0 else imagA
                    for oi in range(6):
                        psl = psglu.tile([128, N], f32, tag="ps")
                        psr = psglu.tile([128, N], f32, tag="ps")
                        for ci in range(ncis):
                            rhs = pts[:, ci, :] if path == 0 else pts[:, 3, :]
                            if path == 0:
                                ll = w["wa_re"][:, 0, ci, oi * 128:(oi + 1) * 128]
                                lr = w["wa_re"][:, 1, ci, oi * 128:(oi + 1) * 128]
                            else:
                                ll = w["wa_im"][:, 0, oi * 128:(oi + 1) * 128]
                                lr = w["wa_im"][:, 1, oi * 128:(oi + 1) * 128]
                            nc.tensor.matmul(psl, lhsT=ll, rhs=rhs,
                                             start=(ci == 0), stop=(ci == ncis - 1))
                            nc.tensor.matmul(psr, lhsT=lr, rhs=rhs,
                                             start=(ci == 0), stop=(ci == ncis - 1))
                        sig = sgp.tile([128, N], f32, tag="sig")
                        nc.scalar.activation(sig, psr, AF.Sigmoid,
                                             bias=w["ba"][:, path, 1, oi:oi + 1])
                        nc.vector.scalar_tensor_tensor(
                            out=dst[:, oi, :], in0=psl,
                            scalar=w["ba"][:, path, 0, oi:oi + 1], in1=sig,
                            op0=OP.add, op1=OP.mult)

                # ---- GLU B1 ----
                realB = bpool.tile([128, 6, N], bf16, tag="realB")
                imagB = bpool.tile([128, 6, N], bf16, tag="imagB")
                for path in range(2):
                    src = realA if path == 0 else imagA
                    dst = realB if path == 0 else imagB
                    ml, mr = (0, 1) if path == 0 else (2, 3)
                    for oi in range(6):
                        psl = psglu.tile([128, N], f32, tag="ps")
                        psr = psglu.tile([128, N], f32, tag="ps")
                        for ci in range(6):
                            rhs = src[:, ci, :]
                            nc.tensor.matmul(
                                psl, lhsT=w["wb1"][:, ml, ci, oi * 128:(oi + 1) * 128],
                                rhs=rhs, start=(ci == 0), stop=(ci == 5))
                            nc.tensor.matmul(
                                psr, lhsT=w["wb1"][:, mr, ci, oi * 128:(oi + 1) * 128],
                                rhs=rhs, start=(ci == 0), stop=(ci == 5))
                        sig = sgp.tile([128, N], f32, tag="sig")
                        nc.scalar.activation(sig, psr, AF.Sigmoid,
                                             bias=w["bb1"][:, mr, oi:oi + 1])
                        nc.vector.scalar_tensor_tensor(
                            out=dst[:, oi, :], in0=psl,
                            scalar=w["bb1"][:, ml, oi:oi + 1], in1=sig,
                            op0=OP.add, op1=OP.mult)

                # ---- GLU B2 (only the output chunks the head consumes) ----
                Rsb = rpool.tile([96, 6, N], f32r, tag="Rsb")
                Isb = rpool.tile([96, 2, N], f32r, tag="Isb")
                for path in range(2):
                    noc = 6 if path == 0 else 2
                    src = realB if path == 0 else imagB
                    wgt = w["wb2r"] if path == 0 else w["wb2i"]
                    dst = Rsb if path == 0 else Isb
                    for oc in range(noc):
                        psl = psglu.tile([96, N], f32, tag="ps")
                        psr = psglu.tile([96, N], f32, tag="ps")
                        for ci in range(6):
                            rhs = src[:, ci, :]
                            nc.tensor.matmul(
                                psl, lhsT=wgt[:, 0, ci, oc * 96:(oc + 1) * 96],
                                rhs=rhs, start=(ci == 0), stop=(ci == 5))
                            nc.tensor.matmul(
                                psr, lhsT=wgt[:, 1, ci, oc * 96:(oc + 1) * 96],
                                rhs=rhs, start=(ci == 0), stop=(ci == 5))
                        boff = oc if path == 0 else 6 + oc
                        sig = sgp.tile([96, N], f32, tag="sig")
                        nc.scalar.activation(sig, psr, AF.Sigmoid,
                                             bias=w["bb2"][:, 1, boff:boff + 1])
                        nc.vector.scalar_tensor_tensor(
                            out=dst[:, oc, :], in0=psl,
                            scalar=w["bb2"][:, 0, boff:boff + 1], in1=sig,
                            op0=OP.add, op1=OP.mult)

                # ---- head (float32r matmuls) ----
                def mmr(ps, lhsT, rhs, start, stop):
                    nc.tensor.matmul(ps, lhsT=lhsT, rhs=rhs, start=start, stop=stop)

                igft = hp.tile([96, 2, N], f32r, tag="igft")
                for ub in range(2):
                    ps = pspt.tile([96, N], f32, tag="ps_pt")
                    k = 0
                    for j in range(3):
                        for tb in range(2):
                            mmr(ps, w["wms"][:, j, tb, ub, :], Rsb[:, 2 * j + tb, :],
                                k == 0, k == 7)
                            k += 1
                    for tb in range(2):
                        mmr(ps, w["wms"][:, 3, tb, ub, :], Isb[:, tb, :], k == 0, k == 7)
                        k += 1
                    nc.scalar.copy(igft[:, ub, :], ps)

                src_sb = hp.tile([96, 2, N], f32r, tag="srcsb")
                for ob in range(2):
                    ps = pspt.tile([96, N], f32, tag="ps_pt")
                    for cb in range(2):
                        mmr(ps, w["wfl"][:, cb, ob, :], igft[:, cb, :], cb == 0, cb == 1)
                    nc.scalar.activation(src_sb[:, ob, :], ps, AF.Sigmoid,
                                         bias=w["bhd"][:, ob:ob + 1])

                fo = hp.tile([96, N], f32r, tag="fo")
                ps = pspt.tile([96, N], f32, tag="ps_pt")
                for cb in range(2):
                    mmr(ps, w["wfrl"][:, cb, :], src_sb[:, cb, :], cb == 0, cb == 1)
                nc.vector.tensor_scalar_add(fo, ps, w["bhd"][:, 2:3])

                h1 = hp.tile([96, N], f32r, tag="h1")
                ps = pspt.tile([96, N], f32, tag="ps_pt")
                mmr(ps, w["wfc1"][:, :], fo[:, :], True, True)
                nc.scalar.activation(h1, ps, AF.Lrelu, bias=w["bhd"][:, 3:4],
                                     alpha=0.01)

                osb = hp.tile([1, N], f32, tag="osb")
                ps2 = pspt.tile([1, N], f32, tag="ps_pt")
                mmr(ps2, w["wfc2"][:, :], h1[:, :], True, True)
                nc.vector.tensor_scalar_add(osb, ps2, w["bfc2"][0:1, 0:1])
                nc.sync.dma_start(out=d_out[b:b + 1, :], in_=osb)

    nc.compile()
    return nc


# ----------------------------------------------------------------------------
# entry point
# ----------------------------------------------------------------------------

def kernel(**inputs):
    global LAST_RESULTS
    from concourse import bass_utils
    import concourse.mybir as mybir

    inputs = {k: np.asarray(v) for k, v in inputs.items()}
    dev, att = _host_precompute(inputs)
    x_shards = _pack_x(inputs["x"].astype(np.float32))

    f32 = mybir.dt.float32
    f32r = mybir.dt.float32r
    bf16 = mybir.dt.bfloat16
    head_r = {"wms", "wfl", "wfrl", "wfc1", "wfc2"}
    dev_shapes = [(k, v.shape,
                   bf16 if v.dtype == BF16 else (f32r if k in head_r else f32))
                  for k, v in dev.items()]

    key = tuple((n, tuple(s), str(dt_)) for n, s, dt_ in dev_shapes)
    if key not in _CACHE:
        _CACHE[key] = _build_program(dev_shapes)
    nc = _CACHE[key]

    in_maps = []
    for c in range(NCORES):
        m = dict(dev)
        m["xt"] = np.ascontiguousarray(x_shards[c])
        in_maps.append(m)

    trace = bool(int(os.environ.get("KERNEL_TRACE", "0")))
    res = bass_utils.run_bass_kernel_spmd(
        nc, in_maps, core_ids=list(range(NCORES)), trace=trace)
    LAST_RESULTS = res

    out = np.concatenate([r["out"] for r in res.results], axis=0)  # (16, 512)
    forecast = out.reshape(B, 1, N).astype(np.float32)
    return forecast, att
